# revision 42
# baseline (speedup 1.0000x reference)
"""Trainium2 Bass kernel for nn_MultiHeadAttention (B=32, S=1024, E=64, H=8, D=64).

Strategy (per core; batch-parallel over 8 cores, 4 batches each):
  - Host-side numpy prep: permute w_qkv columns into per-head Q/K/V blocks,
    transpose x to xT (head-dim on partitions), duplicate operands across
    both 64-partition halves so pairs of K=64 matmuls run as PE row-tile
    pairs. V is pre-scaled by 1/8 (the post-softmax scale). Biases are
    asserted zero (the module spec fills them with zeros) and dropped.
  - On chip, everything stays in "transposed" layouts so no PE transposes
    are needed anywhere:
      qT/kT:   [2 heads * 64 d, 1024 nq]  (4 tiles per batch)
      V:       [128 nk-chunk, 8 heads * (64 v | 1)]  ones col => rowsums
      E^T:     [128 nk, 2 heads * 512 nq] per chunk -> exp -> P^T
      P^T @ [V|1]: accumulates [65, 512] per head: rows 0..63 = (P V)/8,
                   row 64 = rowsum; normalization = multiply by broadcast
                   reciprocal of row 64 (no max subtraction: |E| < ~85 so
                   fp32 exp stays finite; softmax is shift invariant).
      proj:    per-head K=64 matmuls accumulate yT [64 e, nq].
  - The exp stream is SPLIT ACROSS THREE ENGINES to break the ScalarE
    bottleneck: per 8-chunk unit the chunk modes follow `exp_pattern`:
      'S'  : ScalarE ACT Exp psum->sbuf (~1.05us/chunk)
      'DG' : DVE copy psum->sbuf f32 (~0.73us) + GpSimd Schraudolph
             (convert(x*A + B) -> int32, bits reinterpreted as fp32;
             rel err ~1.8% rms, end-to-end ~4e-3) (~0.61us)
      'SG' : ScalarE Copy psum->sbuf + the same GpSimd Schraudolph
    which balances ScalarE/DVE/GpSimd at ~41us each per batch, matching
    the PE's ~42us/batch of matmul streaming.
  - Normalize multiplies run on GpSimd (ov *= recB); qkv/ov/yT PSUM
    evacuations are plain DVE copies; reciprocal on DVE; all DMAs ride
    the two hardware DGE queues (sync/scalar) so GpSimd does no SWDGE.
  - The whole kernel is one software-pipelined stream: eT matmuls run
    `lag` chunks ahead of the PV matmuls; next-batch QKV, normalize
    mults, and projections are injected into fixed chunk slots of later
    units so they never stall the exp chain.
"""

import sys

import numpy as np

_TRN_REPO = "/opt/trn_rl_repo"
if _TRN_REPO not in sys.path:
    sys.path.insert(0, _TRN_REPO)

B, S, E, H, D = 32, 1024, 64, 8, 64
HID = H * D  # 512
N_CORES = 8
NQH = 512  # nq half processed per psum tile

SCH_A = 184.6650292  # 2^7 / ln 2 (bf16-bits Schraudolph)
SCH_B = 16248.625

DEFAULT_PATTERN = ("S", "DG", "S", "DG", "S", "SG", "S", "DG")


def build_nc(bb=B // N_CORES, reps=1, lag=3, mult_depth=2, proj_depth=3,
             dg_slots=(1, 5), dg_extra=None, sg_slots=(), gp_dma=True,
             ov_bf16=False, mult_spread=0, ef_f16=False, v_scalar=False):
    """Build the per-core Bass kernel. bb = batches per core."""
    import concourse.bass as bass
    import concourse.mybir as mybir
    import concourse.tile as tile
    from concourse import bacc
    from contextlib import ExitStack

    f32 = mybir.dt.float32
    f32r = mybir.dt.float32r
    bf16 = mybir.dt.bfloat16
    f16 = mybir.dt.float16
    i16 = mybir.dt.int16
    Exp = mybir.ActivationFunctionType.Exp
    Copy = mybir.ActivationFunctionType.Copy
    Mult = mybir.AluOpType.mult
    Add = mybir.AluOpType.add

    nc = bacc.Bacc(None, target_bir_lowering=False)
    dtp = mybir.dt.bfloat16 if ov_bf16 else f32r

    # ---- DRAM I/O (host-prepped layouts) ----
    xT_d = nc.dram_tensor("xT", [bb, 128, S], f32r, kind="ExternalInput")
    wq_d = nc.dram_tensor("wq", [128, HID], f32r, kind="ExternalInput")
    wk_d = nc.dram_tensor("wk", [128, HID], f32r, kind="ExternalInput")
    wv_d = nc.dram_tensor("wv", [128, HID], f32r, kind="ExternalInput")  # pre /8
    wp_d = nc.dram_tensor("wp", [64, H, E], dtp, kind="ExternalInput")
    yT_d = nc.dram_tensor("yT", [bb, E, S], f32, kind="ExternalOutput")

    with tile.TileContext(nc) as tc, ExitStack() as ctx:
        wpool = ctx.enter_context(tc.tile_pool(name="weights", bufs=1))
        qkpool = ctx.enter_context(tc.tile_pool(name="qk", bufs=2))
        vpool = ctx.enter_context(tc.tile_pool(name="v", bufs=2))
        ptpool = ctx.enter_context(tc.tile_pool(name="pt", bufs=8))
        efpool = ctx.enter_context(tc.tile_pool(name="ef", bufs=4))
        ovpool = ctx.enter_context(tc.tile_pool(name="ov", bufs=16))
        rbpool = ctx.enter_context(tc.tile_pool(name="rb", bufs=10))
        miscpool = ctx.enter_context(tc.tile_pool(name="misc", bufs=2))
        psum_e = ctx.enter_context(tc.tile_pool(name="psum_e", bufs=2, space="PSUM"))
        psum_s = ctx.enter_context(tc.tile_pool(name="psum_s", bufs=4, space="PSUM"))
        drampool = ctx.enter_context(tc.tile_pool(name="dram", bufs=4, space="DRAM"))

        # ---- weights ----
        wq_sb = wpool.tile([128, HID], f32r)
        wk_sb = wpool.tile([128, HID], f32r)
        wv_sb = wpool.tile([128, HID], f32r)
        nc.scalar.dma_start(out=wq_sb[:, 0:256], in_=wq_d[:, 0:256])
        nc.sync.dma_start(out=wk_sb[:, 0:256], in_=wk_d[:, 0:256])

        def load_weight_tail():
            nc.scalar.dma_start(out=wv_sb[:, 0:256], in_=wv_d[:, 0:256])
            nc.sync.dma_start(out=wv_sb[:, 256:512], in_=wv_d[:, 256:512])
            nc.sync.dma_start(out=wq_sb[:, 256:512], in_=wq_d[:, 256:512])
            nc.scalar.dma_start(out=wk_sb[:, 256:512], in_=wk_d[:, 256:512])
            nc.sync.dma_start(out=wp_sb, in_=wp_d[:, :, :])

        wp_sb = wpool.tile([64, H, E], dtp)
        ones64 = wpool.tile([1, 64], f32, name="ones64")
        nc.vector.memset(ones64, 1.0)

        def alloc_batch(bi, b, first=False):
            xT_sb = qkpool.tile([128, S], f32r, tag="xT", name=f"xT_{b}")
            # 4-way column split: first batch all-hardware DGE; later batches
            # ride the gpsimd software DGE for half the columns (bulk,
            # non-critical) to keep the two hardware queues free for the
            # small latency-sensitive rowsum/broadcast chains.
            q2 = nc.scalar if first or not gp_dma else nc.gpsimd
            nc.sync.dma_start(out=xT_sb[:, 0:256], in_=xT_d[bi][:, 0:256])
            q2.dma_start(out=xT_sb[:, 256:512], in_=xT_d[bi][:, 256:512])
            nc.sync.dma_start(out=xT_sb[:, 512:768], in_=xT_d[bi][:, 512:768])
            q2.dma_start(out=xT_sb[:, 768:1024], in_=xT_d[bi][:, 768:1024])
            if first:
                load_weight_tail()
            qT = [qkpool.tile([128, S], f32r, tag=f"qT{t}", name=f"qT{t}_{b}") for t in range(4)]
            kT = [qkpool.tile([128, S], f32r, tag=f"kT{t}", name=f"kT{t}_{b}") for t in range(4)]
            v_nat = [vpool.tile([128, H * 65], bf16, tag=f"v{c}", name=f"v{c}_{b}") for c in range(8)]
            return dict(bi=bi, b=b, xT=xT_sb, qT=qT, kT=kT, v=v_nat,
                        ov={}, oT={}, recB={}, pt={})

        def emit_qk_pair(st, qki, tp, halves=(0, 1)):
            w_sb = (wq_sb, wk_sb)[qki]
            dst = (st["qT"], st["kT"])[qki]
            xT_sb, b = st["xT"], st["b"]
            for half in halves:
                nq = slice(half * NQH, (half + 1) * NQH)
                ps_e = psum_s.tile([128, NQH], f32, tag="small", name=f"psqkv_e{b}_{qki}{tp}{half}")
                ps_o = psum_s.tile([128, NQH], f32, tag="small", name=f"psqkv_o{b}_{qki}{tp}{half}")
                nc.tensor.matmul(ps_e, w_sb[0:64, 128 * tp : 128 * (tp + 1)], xT_sb[0:64, nq])
                nc.tensor.matmul(ps_o, w_sb[64:128, 128 * (tp + 1) : 128 * (tp + 2)], xT_sb[64:128, nq])
                nc.vector.tensor_copy(dst[tp][:, nq], ps_e)
                nc.vector.tensor_copy(dst[tp + 1][:, nq], ps_o)

        def emit_v_pair(st, cp):
            xT_sb, v_nat, b = st["xT"], st["v"], st["b"]
            ps_e = psum_s.tile([128, HID], f32, tag="small", name=f"psv_e{b}_{cp}")
            ps_o = psum_s.tile([128, HID], f32, tag="small", name=f"psv_o{b}_{cp}")
            nc.tensor.matmul(ps_e, xT_sb[0:64, 128 * cp : 128 * (cp + 1)], wv_sb[0:64, :])
            nc.tensor.matmul(ps_o, xT_sb[64:128, 128 * (cp + 1) : 128 * (cp + 2)], wv_sb[64:128, :])
            for c, pss in ((cp, ps_e), (cp + 1, ps_o)):
                vdst = v_nat[c].rearrange("p (h c65) -> p h c65", c65=65)
                if v_scalar:
                    nc.scalar.activation(
                        vdst[:, :, 0:64], pss.rearrange("p (h d) -> p h d", d=64), Copy
                    )
                else:
                    nc.vector.tensor_copy(
                        vdst[:, :, 0:64], pss.rearrange("p (h d) -> p h d", d=64)
                    )
                nc.vector.memset(vdst[:, :, 64], 1.0)

        # ---- software-pipelined attention emission ----
        # PSUM accumulation is order-independent, so PV matmuls for
        # slow-path (DG) chunks are deferred to the end of their unit:
        # the Schraudolph chain (DVE copy -> GpSimd) gets ~6 chunk-times
        # of slack instead of `lag`, so it never stalls the PE.
        pv_q = []  # pending (st, hp, half, c) PV emissions (lag behind eT)
        dg_pending = []  # DG-chunk PVs deferred to unit end
        pv_n = {}  # (id(st), hp, half) -> PV emission counter (start/stop flags)

        def emit_eT(st, hp, half, c):
            qT, kT, b = st["qT"], st["kT"], st["b"]
            nq = slice(half * NQH, (half + 1) * NQH)
            eT = psum_e.tile([128, 2 * NQH], f32, tag="eT", name=f"eT_{b}_{hp}_{half}_{c}")
            nc.tensor.matmul(
                eT[:, 0:NQH], kT[hp][0:64, 128 * c : 128 * (c + 1)], qT[hp][0:64, nq]
            )
            nc.tensor.matmul(
                eT[:, NQH : 2 * NQH],
                kT[hp][64:128, 128 * c : 128 * (c + 1)],
                qT[hp][64:128, nq],
            )
            if c in dg_slots or (dg_extra and c == dg_extra[0] and nunits % dg_extra[1] == 1):
                mode = "DG"
            elif c in sg_slots:
                mode = "SG"
            else:
                mode = "S"
            if mode == "S":
                pt = ptpool.tile([128, 2 * NQH], bf16, tag="pt", name=f"pt_{b}_{hp}_{half}_{c}")
                nc.scalar.activation(pt, eT, Exp)
                st["pt"][(hp, half, c)] = (pt, False)
            else:
                ef = efpool.tile([128, 2 * NQH], f16 if ef_f16 else f32,
                                 tag="ef", name=f"ef_{b}_{hp}_{half}_{c}")
                if mode == "DG":
                    nc.vector.tensor_copy(ef, eT)
                else:
                    nc.scalar.activation(ef, eT, Copy)
                pt = ptpool.tile([128, 2 * NQH], i16, tag="pt", name=f"pt_{b}_{hp}_{half}_{c}")
                nc.gpsimd.tensor_scalar(pt, ef, SCH_A, SCH_B, Mult, Add)
                st["pt"][(hp, half, c)] = (pt, True)
            if mode == "S":
                pv_q.append((st, hp, half, c))
            else:
                dg_pending.append((st, hp, half, c))
            if c == 7:
                pv_q.extend(dg_pending)
                dg_pending.clear()

        def pop_pv():
            st, hp, half, c = pv_q.pop(0)
            b, v_nat = st["b"], st["v"]
            key = (st["b"], hp, half)
            n = pv_n.get(key, 0)
            pv_n[key] = n + 1
            if n == 0:
                st["oT"][(hp, half)] = (
                    psum_s.tile([65, NQH], f32, tag="small", name=f"oTe_{b}_{hp}_{half}"),
                    psum_s.tile([65, NQH], f32, tag="small", name=f"oTo_{b}_{hp}_{half}"),
                )
            oT_e, oT_o = st["oT"][(hp, half)]
            pt, is_int = st["pt"].pop((hp, half, c))
            m0 = pt[:, 0:NQH].bitcast(bf16) if is_int else pt[:, 0:NQH]
            m1 = pt[:, NQH : 2 * NQH].bitcast(bf16) if is_int else pt[:, NQH : 2 * NQH]
            nc.tensor.matmul(
                oT_e,
                v_nat[c][:, (2 * hp) * 65 : (2 * hp) * 65 + 65],
                m0,
                start=(n == 0),
                stop=(n == 7),
            )
            nc.tensor.matmul(
                oT_o,
                v_nat[c][:, (2 * hp + 1) * 65 : (2 * hp + 1) * 65 + 65],
                m1,
                start=(n == 0),
                stop=(n == 7),
            )
            if n == 7:
                finish_unit(st, hp, half)

        def finish_unit(st, hp, half):
            """ov copies, rowsum gather, and the per-pair reciprocal ->
            broadcast chain (all off the critical exp path)."""
            b = st["b"]
            oT_pair = st["oT"].pop((hp, half))
            # the run's very last pair takes the express path: reciprocal
            # straight off the ov rowsum row, broadcast later by a K=1 PE
            # matmul — no DMA round trips
            fast = st.get("_last") and half == 1 and hp == 3
            rs_dram = drampool.tile([2, NQH], dtp, tag="rsd", name=f"rsd_{b}_{hp}_{half}", bufs=4)
            for par, oT in enumerate(oT_pair):
                h = 2 * hp + par
                t = ovpool.tile([65, NQH], dtp, tag="ov", name=f"ov_{b}_{h}_{half}")
                nc.vector.tensor_copy(t, oT)
                st["ov"][(h, half)] = t
                # rowsum row -> DRAM (hop through DRAM for the partition
                # broadcast; SBUF->SBUF DMA misbehaves on hw)
                nc.sync.dma_start(out=rs_dram[par : par + 1, :], in_=t[64:65, :])
            if fast:
                # endgame pair: per-head [1, 512] reciprocal tiles feed K=1
                # broadcast matmuls (moving operands must start at partition 0)
                for par in range(2):
                    h = 2 * hp + par
                    xin = miscpool.tile([1, NQH], dtp, tag=f"xi{par}", name=f"xi_{b}_{h}", bufs=1)
                    nc.sync.dma_start(out=xin, in_=rs_dram[par : par + 1, :])
                    if ov_bf16:
                        xin32 = miscpool.tile([1, NQH], f32, tag=f"xj{par}", name=f"xj_{b}_{h}", bufs=1)
                        nc.vector.tensor_copy(xin32, xin)
                    else:
                        xin32 = xin.bitcast(f32)
                    xr = miscpool.tile([1, NQH], f32, tag=f"xr{par}", name=f"xr_{b}_{h}", bufs=1)
                    xscr = miscpool.tile([1, NQH], f32, tag="xscr", name=f"xscr_{b}_{h}", bufs=1)
                    nc.vector.reciprocal_approx_accurate(xr, xin32, xscr)
                    st.setdefault("xr", {})[(h, half)] = xr
                return
            rs = miscpool.tile([2, NQH], dtp, tag="rsp", name=f"rs_{b}_{hp}_{half}", bufs=2)
            nc.sync.dma_start(out=rs, in_=rs_dram[:, :])
            if ov_bf16:
                rs32 = miscpool.tile([2, NQH], f32, tag="rs32", name=f"rs32_{b}_{hp}_{half}", bufs=2)
                nc.vector.tensor_copy(rs32, rs)
            else:
                rs32 = rs.bitcast(f32)
            rcp = miscpool.tile([2, NQH], f32, tag="rcp", name=f"rcp_{b}_{hp}_{half}", bufs=2)
            rscr = miscpool.tile([2, NQH], f32, tag="rscr", name=f"rscr_{b}_{hp}_{half}", bufs=1)
            nc.vector.reciprocal_approx_accurate(rcp, rs32, rscr)
            rcp_b = miscpool.tile([2, NQH], bf16, tag="rcpb", name=f"rcpb_{b}_{hp}_{half}", bufs=2)
            nc.vector.tensor_copy(rcp_b, rcp)
            rcp_dram = drampool.tile([2, NQH], bf16, tag="rcpd", name=f"rcpd_{b}_{hp}_{half}", bufs=8)
            nc.sync.dma_start(out=rcp_dram, in_=rcp_b)
            for par in range(2):
                h = 2 * hp + par
                recB = rbpool.tile([64, NQH], bf16, tag="recB", name=f"recB_{b}_{h}_{half}")
                nc.sync.dma_start(
                    out=recB, in_=rcp_dram[par : par + 1, :].partition_broadcast(64)
                )
                st["recB"][(h, half)] = recB

        def emit_mults(st, half, heads=range(H), eng=None):
            ov, recB = st["ov"], st["recB"]
            for h in heads:
                (eng or nc.vector).tensor_tensor(
                    ov[(h, half)][0:64, :],
                    ov[(h, half)][0:64, :],
                    recB.pop((h, half)),
                    Mult,
                )

        def emit_proj(st, half, hs=(0, H), tail=False):
            ov, b, bi = st["ov"], st["b"], st["bi"]
            if "yT" not in st:
                st["yT"] = miscpool.tile([E, S], f32, tag="yT", name=f"yTsb_{b}", bufs=2)
            yT_sb = st["yT"]
            nq = slice(half * NQH, (half + 1) * NQH)
            if hs[0] == 0:
                st.setdefault("yT_ps", {})[half] = psum_s.tile(
                    [E, NQH], f32, tag="small", name=f"yTps_{b}_{half}"
                )
            yT_ps = st["yT_ps"][half]
            for h in range(*hs):
                if tail:
                    # endgame: interleave mult h+1 (DVE) with proj h (PE)
                    nc.vector.tensor_tensor(
                        ov[(h, half)][0:64, :],
                        ov[(h, half)][0:64, :],
                        st["recB"].pop((h, half)),
                        Mult,
                    )
                nc.tensor.matmul(
                    yT_ps,
                    wp_sb[:, h, :],
                    ov.pop((h, half))[0:64, :],
                    start=(h == 0),
                    stop=(h == H - 1),
                )
            if hs[1] < H:
                return
            nc.vector.tensor_copy(yT_sb[:, nq], yT_ps)
            # split stores by half and across queues: the final store is small
            dma_a = nc.scalar if tail else nc.sync
            dma_b = nc.scalar if tail else (nc.gpsimd if gp_dma else nc.scalar)
            o0 = half * NQH
            dma_a.dma_start(out=yT_d[bi][:, o0 : o0 + 256], in_=yT_sb[:, o0 : o0 + 256])
            dma_b.dma_start(out=yT_d[bi][:, o0 + 256 : o0 + NQH], in_=yT_sb[:, o0 + 256 : o0 + NQH])

        # ---- schedule ----
        batches = [(rep, bi) for rep in range(reps) for bi in range(bb)]
        sts = {0: alloc_batch(batches[0][1], batches[0][0] * 1000 + batches[0][1], first=True)}
        emit_qk_pair(sts[0], 0, 0, halves=(0,))
        emit_qk_pair(sts[0], 1, 0, halves=(0,))

        mult_due = []  # (unit_idx_done, st, half)
        mult_stream = []  # (st, half, h) ready, spread 2/slot over the unit
        mult_left = {}  # (b, half) -> heads not yet multiplied
        proj_due = []
        nunits = 0

        def drain_mults(limit):
            n = 0
            while mult_stream and n < limit:
                st_m, half_m, h = mult_stream.pop(0)
                emit_mults(st_m, half_m, heads=(h,))
                n += 1
                left = mult_left[(st_m["b"], half_m)]
                left.discard(h)
                if not left:
                    proj_due.append((nunits, st_m, half_m))

        def fillers(i, st, hp, half, c, last):
            """Extra work injected at chunk slot c of unit (hp, half)."""
            prologue = i == 0 and half == 0 and hp == 0
            if prologue:
                # spread the remaining first-batch qkv through unit 0's slots
                if c == 0:
                    emit_v_pair(st, 0)
                    emit_v_pair(st, 2)
                elif c == 1:
                    emit_qk_pair(st, 1, 0, halves=(1,))
                elif c == 2:
                    emit_v_pair(st, 4)
                    emit_v_pair(st, 6)
                elif c == 3:
                    emit_qk_pair(st, 0, 0, halves=(1,))
                elif c == 4 and i + 1 < len(batches):
                    rep, bi = batches[i + 1]
                    sts[i + 1] = alloc_batch(bi, rep * 1000 + bi)
                elif c == 5:
                    emit_qk_pair(st, 0, 2)
                elif c == 6:
                    emit_qk_pair(st, 1, 2)
                return
            if c == 2 and half == 0 and hp == 0 and i + 1 < len(batches):
                rep, bi = batches[i + 1]
                sts[i + 1] = alloc_batch(bi, rep * 1000 + bi)
            if half == 1 and i + 1 < len(batches):
                # next batch's qkv, spread in 2-matmul micro-bursts so the
                # exp cadence never hiccups
                nxt = sts[i + 1]
                if hp == 0:
                    if c == 2:
                        emit_qk_pair(nxt, 0, 0, halves=(0,))
                    elif c == 4:
                        emit_qk_pair(nxt, 0, 0, halves=(1,))
                elif hp == 1:
                    if c == 2:
                        emit_v_pair(nxt, 0)
                    elif c == 4:
                        emit_v_pair(nxt, 2)
                elif hp == 2:
                    if c == 2:
                        emit_v_pair(nxt, 4)
                    elif c == 4:
                        emit_v_pair(nxt, 6)
                else:
                    if c == 1:
                        emit_qk_pair(nxt, 1, 0, halves=(0,))
                    elif c == 2:
                        emit_qk_pair(nxt, 1, 0, halves=(1,))
                    elif c == 3:
                        emit_qk_pair(nxt, 0, 2, halves=(0,))
                    elif c == 4:
                        emit_qk_pair(nxt, 0, 2, halves=(1,))
                    elif c == 6:
                        emit_qk_pair(nxt, 1, 2, halves=(0,))
                    elif c == 7:
                        emit_qk_pair(nxt, 1, 2, halves=(1,))
            if last and half == 1:
                if hp == 2 and c == 3:
                    emit_mults(st, 1, heads=(0, 1))
                elif hp == 3 and c == 1:
                    emit_mults(st, 1, heads=(2, 3))
                elif hp == 3 and c == 3:
                    emit_proj(st, 1, hs=(0, 4))
            if mult_spread:
                while mult_due and (nunits >= mult_due[0][0] + mult_depth or (last and len(mult_due) > 1)):
                    _, st_m, half_m = mult_due.pop(0)
                    if last and half_m == 1 and st_m is st:
                        continue  # final half handled by the endgame path
                    mult_left[(st_m["b"], half_m)] = set(range(H))
                    mult_stream.extend((st_m, half_m, h) for h in range(H))
                drain_mults(8 if last else mult_spread)
            elif c == 5:
                while mult_due and (nunits >= mult_due[0][0] + mult_depth or (last and len(mult_due) > 1)):
                    _, st_m, half_m = mult_due.pop(0)
                    if last and half_m == 1 and st_m is st:
                        continue  # final half handled by the endgame path
                    emit_mults(st_m, half_m)
                    proj_due.append((nunits, st_m, half_m))
            if c in (6, 7):
                hs = (0, 4) if c == 6 else (4, H)
                if proj_due and (nunits >= proj_due[0][0] + (proj_depth - mult_depth) or (last and len(proj_due) > 1)):
                    _, st_p, half_p = proj_due[0]
                    emit_proj(st_p, half_p, hs=hs)
                    if c == 7:
                        proj_due.pop(0)

        for i in range(len(batches)):
            st = sts.pop(i)
            last = i + 1 >= len(batches)
            st["_last"] = last
            for half in (0, 1):
                for hp in range(4):
                    for c in range(8):
                        emit_eT(st, hp, half, c)
                        fillers(i, st, hp, half, c, last)
                        while len(pv_q) > lag:
                            pop_pv()
                    nunits += 1
                mult_due.append((nunits, st, half))

        while pv_q:
            pop_pv()
        while proj_due:
            _, st_p, half_p = proj_due.pop(0)
            emit_proj(st_p, half_p)
        while mult_due:
            _, st_m, half_m = mult_due.pop(0)
            if (st_m.get("_last") and half_m == 1):
                # endgame: mults for pairs 0/1 and proj h0-3 already emitted
                emit_mults(st_m, 1, heads=(4, 5))
                emit_proj(st_m, 1, hs=(4, 6))
                # express broadcast for pair 3: K=1 matmul into a free eT tile
                bc = psum_e.tile([128, 2 * NQH], f32, tag="eT", name="bc_p3")
                for par in range(2):
                    h = 6 + par
                    rb = bc[0:64, par * NQH : (par + 1) * NQH]
                    nc.tensor.matmul(rb, ones64[0:1, :], st_m["xr"][(h, 1)])
                    st_m["recB"][(h, 1)] = rb
                emit_mults(st_m, 1, heads=(6, 7), eng=nc.vector)  # recB in PSUM
                emit_proj(st_m, 1, hs=(6, H))
            else:
                emit_proj(st_m, half_m, tail=True)

    nc.compile()
    return nc


def _round_f32r(a):
    """Round fp32 to fp32r (11-bit mantissa, RNE) so DMA'd operands are
    pre-rounded as the BIR verifier requires for fp32r matmul consumers."""
    u = np.ascontiguousarray(a, np.float32).view(np.uint32)
    r = (u.astype(np.uint64) + 0x7FF + ((u >> 12) & 1)).astype(np.uint32) & np.uint32(
        0xFFFFF000
    )
    return r.view(np.float32)


def prep_inputs(x, w_qkv, b_qkv, w_proj, b_proj, bb=B // N_CORES, n_cores=N_CORES,
                ov_bf16=False):
    """Host-side prep: permute/duplicate weights, transpose x, shard over cores."""
    x = np.asarray(x, np.float32)
    w_qkv = np.asarray(w_qkv, np.float32)
    b_qkv = np.asarray(b_qkv, np.float32)
    w_proj = np.asarray(w_proj, np.float32)
    b_proj = np.asarray(b_proj, np.float32)
    assert not b_qkv.any() and not b_proj.any(), (
        "kernel assumes zero qkv/proj biases (module spec fills them with zeros)"
    )

    W = w_qkv.reshape(E, H, D, 3)
    wq = np.ascontiguousarray(W[..., 0].reshape(E, HID))
    wk = np.ascontiguousarray(W[..., 1].reshape(E, HID))
    wv = np.ascontiguousarray(W[..., 2].reshape(E, HID)) / 8.0
    wq_dup = _round_f32r(np.concatenate([wq, wq], 0))  # [128, 512]
    wk_dup = _round_f32r(np.concatenate([wk, wk], 0))
    wv_dup = _round_f32r(np.concatenate([wv, wv], 0))

    wp = np.ascontiguousarray(w_proj.reshape(H, 64, E).transpose(1, 0, 2))  # [64, H, E]
    if ov_bf16:
        import ml_dtypes

        wp = wp.astype(ml_dtypes.bfloat16)
    else:
        wp = _round_f32r(wp)

    xT = x.transpose(0, 2, 1)  # [B, 64, S]
    xT_dup = _round_f32r(np.ascontiguousarray(np.concatenate([xT, xT], axis=1)))

    shared = {"wq": wq_dup, "wk": wk_dup, "wv": wv_dup, "wp": wp}
    in_maps = []
    for c in range(n_cores):
        m = dict(shared)
        m["xT"] = np.ascontiguousarray(xT_dup[c * bb : (c + 1) * bb])
        in_maps.append(m)
    return in_maps


_CACHE = {}


def run(inputs, trace=False):
    from concourse.bass_utils import run_bass_kernel_spmd

    if "nc" not in _CACHE:
        _CACHE["nc"] = build_nc()
    nc = _CACHE["nc"]
    in_maps = prep_inputs(**inputs)
    res = run_bass_kernel_spmd(nc, in_maps, core_ids=list(range(N_CORES)), trace=trace)
    bb = B // N_CORES
    y = np.concatenate(
        [res.results[c]["yT"].transpose(0, 2, 1) for c in range(N_CORES)], axis=0
    )
    return np.ascontiguousarray(y), res


def kernel(x, w_qkv, b_qkv, w_proj, b_proj):
    y, _ = run(dict(x=x, w_qkv=w_qkv, b_qkv=b_qkv, w_proj=w_proj, b_proj=b_proj))
    return y


# revision 44
# speedup vs baseline: 1.3199x; 1.3199x over previous
"""Trainium2 Bass kernel for nn_MultiHeadAttention (B=32, S=1024, E=64, H=8, D=64).

Strategy (per core; batch-parallel over 8 cores, 4 batches each):
  - Host-side numpy prep: permute w_qkv columns into per-head Q/K/V blocks,
    transpose x to xT (head-dim on partitions), duplicate operands across
    both 64-partition halves so pairs of K=64 matmuls run as PE row-tile
    pairs. V is pre-scaled by 1/8 (the post-softmax scale). Biases are
    asserted zero (the module spec fills them with zeros) and dropped.
  - On chip, everything stays in "transposed" layouts so no PE transposes
    are needed anywhere:
      qT/kT:   [2 heads * 64 d, 1024 nq]  (4 tiles per batch)
      V:       [128 nk-chunk, 8 heads * (64 v | 1)]  ones col => rowsums
      E^T:     [128 nk, 2 heads * 512 nq] per chunk -> exp -> P^T
      P^T @ [V|1]: accumulates [65, 512] per head: rows 0..63 = (P V)/8,
                   row 64 = rowsum; normalization = multiply by broadcast
                   reciprocal of row 64 (no max subtraction: |E| < ~85 so
                   fp32 exp stays finite; softmax is shift invariant).
      proj:    per-head K=64 matmuls accumulate yT [64 e, nq].
  - The exp stream is SPLIT ACROSS THREE ENGINES to break the ScalarE
    bottleneck: per 8-chunk unit the chunk modes follow `exp_pattern`:
      'S'  : ScalarE ACT Exp psum->sbuf (~1.05us/chunk)
      'DG' : DVE copy psum->sbuf f32 (~0.73us) + GpSimd Schraudolph
             (convert(x*A + B) -> int32, bits reinterpreted as fp32;
             rel err ~1.8% rms, end-to-end ~4e-3) (~0.61us)
      'SG' : ScalarE Copy psum->sbuf + the same GpSimd Schraudolph
    which balances ScalarE/DVE/GpSimd at ~41us each per batch, matching
    the PE's ~42us/batch of matmul streaming.
  - Normalize multiplies run on GpSimd (ov *= recB); qkv/ov/yT PSUM
    evacuations are plain DVE copies; reciprocal on DVE; all DMAs ride
    the two hardware DGE queues (sync/scalar) so GpSimd does no SWDGE.
  - The whole kernel is one software-pipelined stream: eT matmuls run
    `lag` chunks ahead of the PV matmuls; next-batch QKV, normalize
    mults, and projections are injected into fixed chunk slots of later
    units so they never stall the exp chain.
"""

import sys

import numpy as np

_TRN_REPO = "/opt/trn_rl_repo"
if _TRN_REPO not in sys.path:
    sys.path.insert(0, _TRN_REPO)

B, S, E, H, D = 32, 1024, 64, 8, 64
HID = H * D  # 512
N_CORES = 8
NQH = 512  # nq half processed per psum tile

SCH_A = 184.6650292  # 2^7 / ln 2 (bf16-bits Schraudolph)
SCH_B = 16248.625

DEFAULT_PATTERN = ("S", "DG", "S", "DG", "S", "SG", "S", "DG")


def build_nc(bb=B // N_CORES, reps=1, lag=3, mult_depth=2, proj_depth=3,
             dg_slots=(1, 5), dg_extra=None, sg_slots=(), gp_dma=True,
             ov_bf16=False, mult_spread=0, ef_f16=False, v_scalar=False,
             pt_bufs=8, ef_bufs=4):
    """Build the per-core Bass kernel. bb = batches per core."""
    import concourse.bass as bass
    import concourse.mybir as mybir
    import concourse.tile as tile
    from concourse import bacc
    from contextlib import ExitStack

    f32 = mybir.dt.float32
    f32r = mybir.dt.float32r
    bf16 = mybir.dt.bfloat16
    f16 = mybir.dt.float16
    i16 = mybir.dt.int16
    Exp = mybir.ActivationFunctionType.Exp
    Copy = mybir.ActivationFunctionType.Copy
    Mult = mybir.AluOpType.mult
    Add = mybir.AluOpType.add

    nc = bacc.Bacc(None, target_bir_lowering=False)
    dtp = mybir.dt.bfloat16 if ov_bf16 else f32r

    # ---- DRAM I/O (host-prepped layouts) ----
    xT_d = nc.dram_tensor("xT", [bb, 128, S], f32r, kind="ExternalInput")
    wq_d = nc.dram_tensor("wq", [128, HID], f32r, kind="ExternalInput")
    wk_d = nc.dram_tensor("wk", [128, HID], f32r, kind="ExternalInput")
    wv_d = nc.dram_tensor("wv", [128, HID], f32r, kind="ExternalInput")  # pre /8
    wp_d = nc.dram_tensor("wp", [64, H, E], dtp, kind="ExternalInput")
    yT_d = nc.dram_tensor("yT", [bb, E, S], f32, kind="ExternalOutput")

    with tile.TileContext(nc) as tc, ExitStack() as ctx:
        wpool = ctx.enter_context(tc.tile_pool(name="weights", bufs=1))
        qkpool = ctx.enter_context(tc.tile_pool(name="qk", bufs=2))
        vpool = ctx.enter_context(tc.tile_pool(name="v", bufs=2))
        ptpool = ctx.enter_context(tc.tile_pool(name="pt", bufs=pt_bufs))
        efpool = ctx.enter_context(tc.tile_pool(name="ef", bufs=ef_bufs))
        ovpool = ctx.enter_context(tc.tile_pool(name="ov", bufs=16))
        rbpool = ctx.enter_context(tc.tile_pool(name="rb", bufs=10))
        miscpool = ctx.enter_context(tc.tile_pool(name="misc", bufs=2))
        psum_e = ctx.enter_context(tc.tile_pool(name="psum_e", bufs=2, space="PSUM"))
        psum_s = ctx.enter_context(tc.tile_pool(name="psum_s", bufs=4, space="PSUM"))
        drampool = ctx.enter_context(tc.tile_pool(name="dram", bufs=4, space="DRAM"))

        # ---- weights ----
        wq_sb = wpool.tile([128, HID], f32r)
        wk_sb = wpool.tile([128, HID], f32r)
        wv_sb = wpool.tile([128, HID], f32r)
        nc.scalar.dma_start(out=wq_sb[:, 0:256], in_=wq_d[:, 0:256])
        nc.sync.dma_start(out=wk_sb[:, 0:256], in_=wk_d[:, 0:256])

        def load_weight_tail():
            nc.scalar.dma_start(out=wv_sb[:, 0:256], in_=wv_d[:, 0:256])
            nc.sync.dma_start(out=wv_sb[:, 256:512], in_=wv_d[:, 256:512])
            nc.sync.dma_start(out=wq_sb[:, 256:512], in_=wq_d[:, 256:512])
            nc.scalar.dma_start(out=wk_sb[:, 256:512], in_=wk_d[:, 256:512])
            nc.sync.dma_start(out=wp_sb, in_=wp_d[:, :, :])

        wp_sb = wpool.tile([64, H, E], dtp)
        ones64 = wpool.tile([1, 64], f32, name="ones64")
        nc.vector.memset(ones64, 1.0)

        def alloc_batch(bi, b, first=False):
            xT_sb = qkpool.tile([128, S], f32r, tag="xT", name=f"xT_{b}")
            # 4-way column split: first batch all-hardware DGE; later batches
            # ride the gpsimd software DGE for half the columns (bulk,
            # non-critical) to keep the two hardware queues free for the
            # small latency-sensitive rowsum/broadcast chains.
            q2 = nc.scalar if first or not gp_dma else nc.gpsimd
            nc.sync.dma_start(out=xT_sb[:, 0:256], in_=xT_d[bi][:, 0:256])
            q2.dma_start(out=xT_sb[:, 256:512], in_=xT_d[bi][:, 256:512])
            nc.sync.dma_start(out=xT_sb[:, 512:768], in_=xT_d[bi][:, 512:768])
            q2.dma_start(out=xT_sb[:, 768:1024], in_=xT_d[bi][:, 768:1024])
            if first:
                load_weight_tail()
            qT = [qkpool.tile([128, S], f32r, tag=f"qT{t}", name=f"qT{t}_{b}") for t in range(4)]
            kT = [qkpool.tile([128, S], f32r, tag=f"kT{t}", name=f"kT{t}_{b}") for t in range(4)]
            v_nat = [vpool.tile([128, H * 65], bf16, tag=f"v{c}", name=f"v{c}_{b}") for c in range(8)]
            return dict(bi=bi, b=b, xT=xT_sb, qT=qT, kT=kT, v=v_nat,
                        ov={}, oT={}, recB={}, pt={})

        def emit_qk_pair(st, qki, tp, halves=(0, 1)):
            w_sb = (wq_sb, wk_sb)[qki]
            dst = (st["qT"], st["kT"])[qki]
            xT_sb, b = st["xT"], st["b"]
            for half in halves:
                nq = slice(half * NQH, (half + 1) * NQH)
                ps_e = psum_s.tile([128, NQH], f32, tag="small", name=f"psqkv_e{b}_{qki}{tp}{half}")
                ps_o = psum_s.tile([128, NQH], f32, tag="small", name=f"psqkv_o{b}_{qki}{tp}{half}")
                nc.tensor.matmul(ps_e, w_sb[0:64, 128 * tp : 128 * (tp + 1)], xT_sb[0:64, nq])
                nc.tensor.matmul(ps_o, w_sb[64:128, 128 * (tp + 1) : 128 * (tp + 2)], xT_sb[64:128, nq])
                nc.vector.tensor_copy(dst[tp][:, nq], ps_e)
                nc.vector.tensor_copy(dst[tp + 1][:, nq], ps_o)

        def emit_v_pair(st, cp):
            xT_sb, v_nat, b = st["xT"], st["v"], st["b"]
            ps_e = psum_s.tile([128, HID], f32, tag="small", name=f"psv_e{b}_{cp}")
            ps_o = psum_s.tile([128, HID], f32, tag="small", name=f"psv_o{b}_{cp}")
            nc.tensor.matmul(ps_e, xT_sb[0:64, 128 * cp : 128 * (cp + 1)], wv_sb[0:64, :])
            nc.tensor.matmul(ps_o, xT_sb[64:128, 128 * (cp + 1) : 128 * (cp + 2)], wv_sb[64:128, :])
            for c, pss in ((cp, ps_e), (cp + 1, ps_o)):
                vdst = v_nat[c].rearrange("p (h c65) -> p h c65", c65=65)
                if v_scalar:
                    nc.scalar.activation(
                        vdst[:, :, 0:64], pss.rearrange("p (h d) -> p h d", d=64), Copy
                    )
                else:
                    nc.vector.tensor_copy(
                        vdst[:, :, 0:64], pss.rearrange("p (h d) -> p h d", d=64)
                    )
                nc.vector.memset(vdst[:, :, 64], 1.0)

        # ---- software-pipelined attention emission ----
        # PSUM accumulation is order-independent, so PV matmuls for
        # slow-path (DG) chunks are deferred to the end of their unit:
        # the Schraudolph chain (DVE copy -> GpSimd) gets ~6 chunk-times
        # of slack instead of `lag`, so it never stalls the PE.
        pv_q = []  # pending (st, hp, half, c) PV emissions (lag behind eT)
        dg_pending = []  # DG-chunk PVs deferred to unit end
        pv_n = {}  # (id(st), hp, half) -> PV emission counter (start/stop flags)

        def emit_eT(st, hp, half, c):
            qT, kT, b = st["qT"], st["kT"], st["b"]
            nq = slice(half * NQH, (half + 1) * NQH)
            eT = psum_e.tile([128, 2 * NQH], f32, tag="eT", name=f"eT_{b}_{hp}_{half}_{c}")
            nc.tensor.matmul(
                eT[:, 0:NQH], kT[hp][0:64, 128 * c : 128 * (c + 1)], qT[hp][0:64, nq]
            )
            nc.tensor.matmul(
                eT[:, NQH : 2 * NQH],
                kT[hp][64:128, 128 * c : 128 * (c + 1)],
                qT[hp][64:128, nq],
            )
            if c in dg_slots or (dg_extra and c == dg_extra[0] and nunits % dg_extra[1] == 1):
                mode = "DG"
            elif c in sg_slots:
                mode = "SG"
            else:
                mode = "S"
            if mode == "S":
                pt = ptpool.tile([128, 2 * NQH], bf16, tag="pt", name=f"pt_{b}_{hp}_{half}_{c}")
                nc.scalar.activation(pt, eT, Exp)
                st["pt"][(hp, half, c)] = (pt, False)
            else:
                ef = efpool.tile([128, 2 * NQH], f16 if ef_f16 else f32,
                                 tag="ef", name=f"ef_{b}_{hp}_{half}_{c}")
                if mode == "DG":
                    nc.vector.tensor_copy(ef, eT)
                else:
                    nc.scalar.activation(ef, eT, Copy)
                pt = ptpool.tile([128, 2 * NQH], i16, tag="pt", name=f"pt_{b}_{hp}_{half}_{c}")
                nc.gpsimd.tensor_scalar(pt, ef, SCH_A, SCH_B, Mult, Add)
                st["pt"][(hp, half, c)] = (pt, True)
            if mode == "S":
                pv_q.append((st, hp, half, c))
            else:
                dg_pending.append((st, hp, half, c))
            if c == 7:
                pv_q.extend(dg_pending)
                dg_pending.clear()

        def pop_pv():
            st, hp, half, c = pv_q.pop(0)
            b, v_nat = st["b"], st["v"]
            key = (st["b"], hp, half)
            n = pv_n.get(key, 0)
            pv_n[key] = n + 1
            if n == 0:
                st["oT"][(hp, half)] = (
                    psum_s.tile([65, NQH], f32, tag="small", name=f"oTe_{b}_{hp}_{half}"),
                    psum_s.tile([65, NQH], f32, tag="small", name=f"oTo_{b}_{hp}_{half}"),
                )
            oT_e, oT_o = st["oT"][(hp, half)]
            pt, is_int = st["pt"].pop((hp, half, c))
            m0 = pt[:, 0:NQH].bitcast(bf16) if is_int else pt[:, 0:NQH]
            m1 = pt[:, NQH : 2 * NQH].bitcast(bf16) if is_int else pt[:, NQH : 2 * NQH]
            nc.tensor.matmul(
                oT_e,
                v_nat[c][:, (2 * hp) * 65 : (2 * hp) * 65 + 65],
                m0,
                start=(n == 0),
                stop=(n == 7),
            )
            nc.tensor.matmul(
                oT_o,
                v_nat[c][:, (2 * hp + 1) * 65 : (2 * hp + 1) * 65 + 65],
                m1,
                start=(n == 0),
                stop=(n == 7),
            )
            if n == 7:
                finish_unit(st, hp, half)

        def finish_unit(st, hp, half):
            """ov copies, rowsum gather, and the per-pair reciprocal ->
            broadcast chain (all off the critical exp path)."""
            b = st["b"]
            oT_pair = st["oT"].pop((hp, half))
            # the run's very last pair takes the express path: reciprocal
            # straight off the ov rowsum row, broadcast later by a K=1 PE
            # matmul — no DMA round trips
            fast = st.get("_last") and half == 1 and hp == 3
            rs_dram = drampool.tile([2, NQH], dtp, tag="rsd", name=f"rsd_{b}_{hp}_{half}", bufs=4)
            for par, oT in enumerate(oT_pair):
                h = 2 * hp + par
                t = ovpool.tile([65, NQH], dtp, tag="ov", name=f"ov_{b}_{h}_{half}")
                nc.vector.tensor_copy(t, oT)
                st["ov"][(h, half)] = t
                # rowsum row -> DRAM (hop through DRAM for the partition
                # broadcast; SBUF->SBUF DMA misbehaves on hw)
                nc.sync.dma_start(out=rs_dram[par : par + 1, :], in_=t[64:65, :])
            if fast:
                # endgame pair: per-head [1, 512] reciprocal tiles feed K=1
                # broadcast matmuls (moving operands must start at partition 0)
                for par in range(2):
                    h = 2 * hp + par
                    xin = miscpool.tile([1, NQH], dtp, tag=f"xi{par}", name=f"xi_{b}_{h}", bufs=1)
                    nc.sync.dma_start(out=xin, in_=rs_dram[par : par + 1, :])
                    if ov_bf16:
                        xin32 = miscpool.tile([1, NQH], f32, tag=f"xj{par}", name=f"xj_{b}_{h}", bufs=1)
                        nc.vector.tensor_copy(xin32, xin)
                    else:
                        xin32 = xin.bitcast(f32)
                    xr = miscpool.tile([1, NQH], f32, tag=f"xr{par}", name=f"xr_{b}_{h}", bufs=1)
                    xscr = miscpool.tile([1, NQH], f32, tag="xscr", name=f"xscr_{b}_{h}", bufs=1)
                    nc.vector.reciprocal_approx_accurate(xr, xin32, xscr)
                    st.setdefault("xr", {})[(h, half)] = xr
                return
            rs = miscpool.tile([2, NQH], dtp, tag="rsp", name=f"rs_{b}_{hp}_{half}", bufs=2)
            nc.sync.dma_start(out=rs, in_=rs_dram[:, :])
            if ov_bf16:
                rs32 = miscpool.tile([2, NQH], f32, tag="rs32", name=f"rs32_{b}_{hp}_{half}", bufs=2)
                nc.vector.tensor_copy(rs32, rs)
            else:
                rs32 = rs.bitcast(f32)
            rcp = miscpool.tile([2, NQH], f32, tag="rcp", name=f"rcp_{b}_{hp}_{half}", bufs=2)
            rscr = miscpool.tile([2, NQH], f32, tag="rscr", name=f"rscr_{b}_{hp}_{half}", bufs=1)
            nc.vector.reciprocal_approx_accurate(rcp, rs32, rscr)
            rcp_b = miscpool.tile([2, NQH], bf16, tag="rcpb", name=f"rcpb_{b}_{hp}_{half}", bufs=2)
            nc.vector.tensor_copy(rcp_b, rcp)
            rcp_dram = drampool.tile([2, NQH], bf16, tag="rcpd", name=f"rcpd_{b}_{hp}_{half}", bufs=8)
            nc.sync.dma_start(out=rcp_dram, in_=rcp_b)
            for par in range(2):
                h = 2 * hp + par
                recB = rbpool.tile([64, NQH], bf16, tag="recB", name=f"recB_{b}_{h}_{half}")
                nc.sync.dma_start(
                    out=recB, in_=rcp_dram[par : par + 1, :].partition_broadcast(64)
                )
                st["recB"][(h, half)] = recB

        def emit_mults(st, half, heads=range(H), eng=None):
            ov, recB = st["ov"], st["recB"]
            for h in heads:
                (eng or nc.vector).tensor_tensor(
                    ov[(h, half)][0:64, :],
                    ov[(h, half)][0:64, :],
                    recB.pop((h, half)),
                    Mult,
                )

        def emit_proj(st, half, hs=(0, H), tail=False):
            ov, b, bi = st["ov"], st["b"], st["bi"]
            if "yT" not in st:
                st["yT"] = miscpool.tile([E, S], f32, tag="yT", name=f"yTsb_{b}", bufs=2)
            yT_sb = st["yT"]
            nq = slice(half * NQH, (half + 1) * NQH)
            if hs[0] == 0:
                st.setdefault("yT_ps", {})[half] = psum_s.tile(
                    [E, NQH], f32, tag="small", name=f"yTps_{b}_{half}"
                )
            yT_ps = st["yT_ps"][half]
            for h in range(*hs):
                if tail:
                    # endgame: interleave mult h+1 (DVE) with proj h (PE)
                    nc.vector.tensor_tensor(
                        ov[(h, half)][0:64, :],
                        ov[(h, half)][0:64, :],
                        st["recB"].pop((h, half)),
                        Mult,
                    )
                nc.tensor.matmul(
                    yT_ps,
                    wp_sb[:, h, :],
                    ov.pop((h, half))[0:64, :],
                    start=(h == 0),
                    stop=(h == H - 1),
                )
            if hs[1] < H:
                return
            nc.vector.tensor_copy(yT_sb[:, nq], yT_ps)
            # split stores by half and across queues: the final store is small
            dma_a = nc.scalar if tail else nc.sync
            dma_b = nc.scalar if tail else (nc.gpsimd if gp_dma else nc.scalar)
            o0 = half * NQH
            dma_a.dma_start(out=yT_d[bi][:, o0 : o0 + 256], in_=yT_sb[:, o0 : o0 + 256])
            dma_b.dma_start(out=yT_d[bi][:, o0 + 256 : o0 + NQH], in_=yT_sb[:, o0 + 256 : o0 + NQH])

        # ---- schedule ----
        batches = [(rep, bi) for rep in range(reps) for bi in range(bb)]
        sts = {0: alloc_batch(batches[0][1], batches[0][0] * 1000 + batches[0][1], first=True)}
        emit_qk_pair(sts[0], 0, 0, halves=(0,))
        emit_qk_pair(sts[0], 1, 0, halves=(0,))

        mult_due = []  # (unit_idx_done, st, half)
        mult_stream = []  # (st, half, h) ready, spread 2/slot over the unit
        mult_left = {}  # (b, half) -> heads not yet multiplied
        proj_due = []
        nunits = 0

        def drain_mults(limit):
            n = 0
            while mult_stream and n < limit:
                st_m, half_m, h = mult_stream.pop(0)
                emit_mults(st_m, half_m, heads=(h,))
                n += 1
                left = mult_left[(st_m["b"], half_m)]
                left.discard(h)
                if not left:
                    proj_due.append((nunits, st_m, half_m))

        def fillers(i, st, hp, half, c, last):
            """Extra work injected at chunk slot c of unit (hp, half)."""
            prologue = i == 0 and half == 0 and hp == 0
            if prologue:
                # spread the remaining first-batch qkv through unit 0's slots
                if c == 0:
                    emit_v_pair(st, 0)
                    emit_v_pair(st, 2)
                elif c == 1:
                    emit_qk_pair(st, 1, 0, halves=(1,))
                elif c == 2:
                    emit_v_pair(st, 4)
                    emit_v_pair(st, 6)
                elif c == 3:
                    emit_qk_pair(st, 0, 0, halves=(1,))
                elif c == 4 and i + 1 < len(batches):
                    rep, bi = batches[i + 1]
                    sts[i + 1] = alloc_batch(bi, rep * 1000 + bi)
                elif c == 5:
                    emit_qk_pair(st, 0, 2)
                elif c == 6:
                    emit_qk_pair(st, 1, 2)
                return
            if c == 2 and half == 0 and hp == 0 and i + 1 < len(batches):
                rep, bi = batches[i + 1]
                sts[i + 1] = alloc_batch(bi, rep * 1000 + bi)
            if half == 1 and i + 1 < len(batches):
                # next batch's qkv, spread in 2-matmul micro-bursts so the
                # exp cadence never hiccups
                nxt = sts[i + 1]
                if hp == 0:
                    if c == 2:
                        emit_qk_pair(nxt, 0, 0, halves=(0,))
                    elif c == 4:
                        emit_qk_pair(nxt, 0, 0, halves=(1,))
                elif hp == 1:
                    if c == 2:
                        emit_v_pair(nxt, 0)
                    elif c == 4:
                        emit_v_pair(nxt, 2)
                elif hp == 2:
                    if c == 2:
                        emit_v_pair(nxt, 4)
                    elif c == 4:
                        emit_v_pair(nxt, 6)
                else:
                    if c == 1:
                        emit_qk_pair(nxt, 1, 0, halves=(0,))
                    elif c == 2:
                        emit_qk_pair(nxt, 1, 0, halves=(1,))
                    elif c == 3:
                        emit_qk_pair(nxt, 0, 2, halves=(0,))
                    elif c == 4:
                        emit_qk_pair(nxt, 0, 2, halves=(1,))
                    elif c == 6:
                        emit_qk_pair(nxt, 1, 2, halves=(0,))
                    elif c == 7:
                        emit_qk_pair(nxt, 1, 2, halves=(1,))
            if last and half == 1:
                if hp == 2 and c == 3:
                    emit_mults(st, 1, heads=(0, 1))
                elif hp == 3 and c == 1:
                    emit_mults(st, 1, heads=(2, 3))
                elif hp == 3 and c == 3:
                    emit_proj(st, 1, hs=(0, 4))
            if mult_spread:
                while mult_due and (nunits >= mult_due[0][0] + mult_depth or (last and len(mult_due) > 1)):
                    _, st_m, half_m = mult_due.pop(0)
                    if last and half_m == 1 and st_m is st:
                        continue  # final half handled by the endgame path
                    mult_left[(st_m["b"], half_m)] = set(range(H))
                    mult_stream.extend((st_m, half_m, h) for h in range(H))
                drain_mults(8 if last else mult_spread)
            elif c == 5:
                while mult_due and (nunits >= mult_due[0][0] + mult_depth or (last and len(mult_due) > 1)):
                    _, st_m, half_m = mult_due.pop(0)
                    if last and half_m == 1 and st_m is st:
                        continue  # final half handled by the endgame path
                    emit_mults(st_m, half_m)
                    proj_due.append((nunits, st_m, half_m))
            if c in (6, 7):
                hs = (0, 4) if c == 6 else (4, H)
                if proj_due and (nunits >= proj_due[0][0] + (proj_depth - mult_depth) or (last and len(proj_due) > 1)):
                    _, st_p, half_p = proj_due[0]
                    emit_proj(st_p, half_p, hs=hs)
                    if c == 7:
                        proj_due.pop(0)

        for i in range(len(batches)):
            st = sts.pop(i)
            last = i + 1 >= len(batches)
            st["_last"] = last
            for half in (0, 1):
                for hp in range(4):
                    for c in range(8):
                        emit_eT(st, hp, half, c)
                        fillers(i, st, hp, half, c, last)
                        while len(pv_q) > lag:
                            pop_pv()
                    nunits += 1
                mult_due.append((nunits, st, half))

        while pv_q:
            pop_pv()
        while proj_due:
            _, st_p, half_p = proj_due.pop(0)
            emit_proj(st_p, half_p)
        while mult_due:
            _, st_m, half_m = mult_due.pop(0)
            if (st_m.get("_last") and half_m == 1):
                # endgame: mults for pairs 0/1 and proj h0-3 already emitted
                emit_mults(st_m, 1, heads=(4, 5))
                emit_proj(st_m, 1, hs=(4, 6))
                # express broadcast for pair 3: K=1 matmul into a free eT tile
                bc = psum_e.tile([128, 2 * NQH], f32, tag="eT", name="bc_p3")
                for par in range(2):
                    h = 6 + par
                    rb = bc[0:64, par * NQH : (par + 1) * NQH]
                    nc.tensor.matmul(rb, ones64[0:1, :], st_m["xr"][(h, 1)])
                    st_m["recB"][(h, 1)] = rb
                emit_mults(st_m, 1, heads=(6, 7), eng=nc.vector)  # recB in PSUM
                emit_proj(st_m, 1, hs=(6, H))
            else:
                emit_proj(st_m, half_m, tail=True)

    nc.compile()
    return nc


def _round_f32r(a):
    """Round fp32 to fp32r (11-bit mantissa, RNE) so DMA'd operands are
    pre-rounded as the BIR verifier requires for fp32r matmul consumers."""
    u = np.ascontiguousarray(a, np.float32).view(np.uint32)
    r = (u.astype(np.uint64) + 0x7FF + ((u >> 12) & 1)).astype(np.uint32) & np.uint32(
        0xFFFFF000
    )
    return r.view(np.float32)


def prep_inputs(x, w_qkv, b_qkv, w_proj, b_proj, bb=B // N_CORES, n_cores=N_CORES,
                ov_bf16=False):
    """Host-side prep: permute/duplicate weights, transpose x, shard over cores."""
    x = np.asarray(x, np.float32)
    w_qkv = np.asarray(w_qkv, np.float32)
    b_qkv = np.asarray(b_qkv, np.float32)
    w_proj = np.asarray(w_proj, np.float32)
    b_proj = np.asarray(b_proj, np.float32)
    assert not b_qkv.any() and not b_proj.any(), (
        "kernel assumes zero qkv/proj biases (module spec fills them with zeros)"
    )

    W = w_qkv.reshape(E, H, D, 3)
    wq = np.ascontiguousarray(W[..., 0].reshape(E, HID))
    wk = np.ascontiguousarray(W[..., 1].reshape(E, HID))
    wv = np.ascontiguousarray(W[..., 2].reshape(E, HID)) / 8.0
    wq_dup = _round_f32r(np.concatenate([wq, wq], 0))  # [128, 512]
    wk_dup = _round_f32r(np.concatenate([wk, wk], 0))
    wv_dup = _round_f32r(np.concatenate([wv, wv], 0))

    wp = np.ascontiguousarray(w_proj.reshape(H, 64, E).transpose(1, 0, 2))  # [64, H, E]
    if ov_bf16:
        import ml_dtypes

        wp = wp.astype(ml_dtypes.bfloat16)
    else:
        wp = _round_f32r(wp)

    xT = x.transpose(0, 2, 1)  # [B, 64, S]
    xT_dup = _round_f32r(np.ascontiguousarray(np.concatenate([xT, xT], axis=1)))

    shared = {"wq": wq_dup, "wk": wk_dup, "wv": wv_dup, "wp": wp}
    in_maps = []
    for c in range(n_cores):
        m = dict(shared)
        m["xT"] = np.ascontiguousarray(xT_dup[c * bb : (c + 1) * bb])
        in_maps.append(m)
    return in_maps


_CACHE = {}


def run(inputs, trace=False):
    from concourse.bass_utils import run_bass_kernel_spmd

    if "nc" not in _CACHE:
        _CACHE["nc"] = build_nc()
    nc = _CACHE["nc"]
    in_maps = prep_inputs(**inputs)
    res = run_bass_kernel_spmd(nc, in_maps, core_ids=list(range(N_CORES)), trace=trace)
    bb = B // N_CORES
    y = np.concatenate(
        [res.results[c]["yT"].transpose(0, 2, 1) for c in range(N_CORES)], axis=0
    )
    return np.ascontiguousarray(y), res


def kernel(x, w_qkv, b_qkv, w_proj, b_proj):
    y, _ = run(dict(x=x, w_qkv=w_qkv, b_qkv=b_qkv, w_proj=w_proj, b_proj=b_proj))
    return y
